# revision 36
# baseline (speedup 1.0000x reference)
"""Additive attention (Bahdanau) on 8 TRN2 NeuronCores.

Full-problem shapes: query [4,512,512], key/value [4,512,512],
Wq/Wk [512,256], bq/bk [256], wv [256], bv [].

  q = query @ Wq + bq                       # [B,Q,H]
  k = key @ Wk + bk                         # [B,K,H]
  score[b,q,k] = wv . tanh(q[b,q]+k[b,k])   # (+bv, dropped: softmax-invariant)
  attn = softmax(score, axis=-1)
  context = attn @ value

Sharding: data-parallel over (batch, query-half): core c handles batch c//2,
query rows (c%2)*256:(c%2+1)*256. Each core sees its full key/value batch, so
softmax is core-local; gather is pure numpy concatenation.

Per-core kernel layout: h (hidden) on partitions. For each query row r, the
scalar engine computes feat = tanh(kTp + qTp[:, r]) as one [128h, 512k]
activation per h-chunk (per-partition bias = q values), the tensor engine then
contracts with wv (feat stationary, wv the 1-column moving operand) writing a
scoreT[k-partition, r] column into PSUM. Softmax runs on the transposed scores
without any max-subtraction (|score| <= sum|wv| ~ 13, safe in fp32): exp on
the scalar engine, key-sum via ones-vector matmul over partitions, reciprocal
broadcast across partitions by a stride-0 DMA. The normalized attnT is
directly the lhsT of the context matmul. attnT is un-transposed on the host.
"""

import numpy as np

import concourse.bass as bass
import concourse.tile as tile
from concourse import bacc, mybir
from concourse.bass_utils import run_bass_kernel_spmd
from concourse.masks import make_identity

F32 = mybir.dt.float32
F16 = mybir.dt.float16

P = 128          # partitions
D = 512          # DQ = DK (projection input dim)
H = 256          # hidden dim; HC = H // P h-chunks
K = 512          # keys per batch; KC = K // P key chunks
QS = 256         # query rows per core
DV = 512         # value dim
HC, KC, DC, QT = H // P, K // P, D // P, QS // P

N_CORES = 8
B, Q = 4, 512


def _build_tile_kernel(tc, ins, outs, n_rows=QS):
    nc = tc.nc
    query, key, value, Wq, bq, Wk, bk, wv = ins
    ctx_out, attnT_out = outs

    raw_pool_cm = tc.tile_pool(name="raw", bufs=1)
    with tc.tile_pool(name="const", bufs=1) as const, \
         tc.tile_pool(name="proj", bufs=1) as proj, \
         tc.tile_pool(name="feat", bufs=2) as featp, \
         tc.tile_pool(name="tailp", bufs=1) as tailp, \
         tc.tile_pool(name="outp", bufs=2) as outp:

        raw = raw_pool_cm.__enter__()
        # ---- input DMAs, critical-path first: key, Wk, query, Wq -------
        k_raw = raw.tile([P, KC, D], F32)
        key_r = key.rearrange("(t p) d -> p t d", p=P)
        for t in range(KC):
            nc.sync.dma_start(k_raw[:, t, :], key_r[:, t, :])
        wk_sb = raw.tile([P, DC, H], F32)
        nc.sync.dma_start(wk_sb[:], Wk.rearrange("(c p) h -> p c h", p=P))
        q_raw = raw.tile([P, QT, D], F32)
        query_r = query.rearrange("(t p) d -> p t d", p=P)
        for t in range(QT):
            nc.sync.dma_start(q_raw[:, t, :], query_r[:, t, :])
        wq_sb = raw.tile([P, DC, H], F32)
        nc.sync.dma_start(wq_sb[:], Wq.rearrange("(c p) h -> p c h", p=P))
        # small/late tensors ride the gpsimd DMA queue, off the critical path
        bq_sb = const.tile([P, HC], F32)
        nc.gpsimd.dma_start(bq_sb[:], bq.rearrange("(o p) -> p o", p=P))
        bk_sb = const.tile([P, HC], F32)
        nc.gpsimd.dma_start(bk_sb[:], bk.rearrange("(o p) -> p o", p=P))
        wv32 = const.tile([P, HC], F32)
        nc.gpsimd.dma_start(wv32[:], wv.rearrange("(o p) -> p o", p=P))
        v_sb = const.tile([P, KC, DV], F32)   # only needed in the tail
        val_r = value.rearrange("(c p) v -> p c v", p=P)
        with tc.tile_wait_until(0.055):  # keep value traffic out of startup
            for t in range(KC):
                nc.gpsimd.dma_start(v_sb[:, t, :], val_r[:, t, :])

        ident = const.tile([P, P], F16)
        make_identity(nc, ident[:])
        # fp16 copies for transposes/projections on the (idle) scalar engine
        k16_raw = const.tile([P, KC, D], F16)
        for t in range(KC):
            nc.scalar.copy(k16_raw[:, t, :], k_raw[:, t, :])
        wk16 = const.tile([P, DC, H], F16)
        nc.scalar.copy(wk16[:], wk_sb[:])
        q16_raw = const.tile([P, QT, D], F16)
        for t in range(QT):
            nc.scalar.copy(q16_raw[:, t, :], q_raw[:, t, :])
        wq16 = const.tile([P, DC, H], F16)
        nc.scalar.copy(wq16[:], wq_sb[:])
        # off the critical path: wv cast + tail constants on gpsimd
        wv16 = const.tile([P, HC], F16)
        nc.gpsimd.tensor_copy(wv16[:], wv32[:])
        ones_sb = const.tile([P, 1], F32)     # k-sum matmul lhsT
        nc.gpsimd.memset(ones_sb[:], 1.0)
        ones_row = const.tile([1, P], F32)    # partition-broadcast via PE
        nc.gpsimd.memset(ones_row[:], 1.0)

        # ---- transpose query/key so d sits on partitions (fp16) --------
        qT = proj.tile([P, DC, QS], F16)      # [d_inner, d_chunk, q]
        kT = proj.tile([P, DC, K], F16)
        qTp = proj.tile([P, HC, QS], F32)     # fp32: feeds tensor_scalar adds
        kTp16 = proj.tile([P, HC, K], F16)
        with tc.tile_pool(name="ps_mm", bufs=2, space="PSUM") as ps_mm:
            for t in range(KC):
                for c in range(DC):
                    pst = ps_mm.tile([P, P], F16, tag="tp")
                    nc.tensor.transpose(pst[:], k16_raw[:, t, c * P:(c + 1) * P],
                                        ident[:])
                    nc.vector.tensor_copy(kT[:, c, t * P:(t + 1) * P], pst[:])
            for t in range(QT):
                for c in range(DC):
                    pst = ps_mm.tile([P, P], F16, tag="tp")
                    nc.tensor.transpose(pst[:], q16_raw[:, t, c * P:(c + 1) * P],
                                        ident[:])
                    nc.vector.tensor_copy(qT[:, c, t * P:(t + 1) * P], pst[:])

            # ---- projections, already transposed: [h, q] and [h, k] ----
            for hs in range(HC):
                psk = ps_mm.tile([P, K], F32, tag="psk", bufs=1)
                for c in range(DC):
                    nc.tensor.matmul(psk[:], wk16[:, c, hs * P:(hs + 1) * P],
                                     kT[:, c, :], start=(c == 0), stop=(c == DC - 1))
                nc.vector.tensor_scalar_add(kTp16[:, hs, :], psk[:],
                                            bk_sb[:, hs:hs + 1])
                psq = ps_mm.tile([P, QS], F32, tag="psq", bufs=1)
                for c in range(DC):
                    nc.tensor.matmul(psq[:], wq16[:, c, hs * P:(hs + 1) * P],
                                     qT[:, c, :], start=(c == 0), stop=(c == DC - 1))
                nc.vector.tensor_scalar_add(qTp[:, hs, :], psq[:],
                                            bq_sb[:, hs:hs + 1])

        raw_pool_cm.__exit__(None, None, None)

        # ---- main loop: per group, DVE adds -> one big tanh ->
        #      per-row wv matvecs into transposed-score PSUM columns.
        #      Small edge groups cut first-act latency and the last
        #      matvec burst before the tail. ----------------------------
        G = 32
        if n_rows == QS:
            group_rows = [8, 8] + [32] * 7 + [8, 8]
        else:
            group_rows = [min(G, n_rows - s0) for s0 in range(0, n_rows, G)]
        with tc.tile_pool(name="ps_score", bufs=1, space="PSUM") as ps_score, \
             tc.tile_pool(name="ps_tail", bufs=1, space="PSUM") as ps_tail, \
             tc.tile_pool(name="sump", bufs=2) as sump:
            score_ps = [ps_score.tile([P, HC, QS], F32, name=f"score_{kc}")
                        for kc in range(KC)]
            ssum = tailp.tile([P, KC, QS], F32)
            expT = tailp.tile([P, KC, QS], F32)
            sums_ps = ps_tail.tile([P, QS], F32, tag="sums")
            attnT = tailp.tile([P, KC, QS], F32)
            recipB = tailp.tile([P, QS], F32)

            TW = 64  # tail chunk width (query columns)

            def tail_part(t):
                """softmax + context for query columns [t*TW, (t+1)*TW)."""
                cs = slice(t * TW, (t + 1) * TW)
                for kc in range(KC):
                    nc.vector.tensor_reduce(
                        ssum[:, kc, cs],
                        score_ps[kc][:, :, cs].rearrange("p h c -> p c h"),
                        axis=mybir.AxisListType.X, op=mybir.AluOpType.add)
                for kc in range(KC):
                    nc.scalar.activation(expT[:, kc, cs], ssum[:, kc, cs],
                                         mybir.ActivationFunctionType.Exp)
                for kc in range(KC):
                    nc.tensor.matmul(sums_ps[0:1, cs], ones_sb[:],
                                     expT[:, kc, cs],
                                     start=(kc == 0), stop=(kc == KC - 1))
                sums_sb = tailp.tile([1, TW], F32, tag="sums_sb", bufs=2,
                                     name="sums_sb")
                nc.vector.tensor_copy(sums_sb[:], sums_ps[0:1, cs])
                # broadcast row across partitions via rank-1 PE outer product
                bc_ps = ps_tail.tile([P, TW], F32, tag="bc", bufs=1)
                nc.tensor.matmul(bc_ps[:], ones_row[:], sums_sb[:],
                                 start=True, stop=True)
                nc.vector.reciprocal(recipB[:, cs], bc_ps[:])
                nc.gpsimd.tensor_tensor(
                    attnT[:, :, cs], expT[:, :, cs],
                    recipB[:, None, cs].to_broadcast((P, KC, TW)),
                    mybir.AluOpType.mult)
                nc.sync.dma_start(
                    attnT_out.rearrange("(c p) q -> p c q", p=P)[:, :, cs],
                    attnT[:, :, cs])
                psc = ps_tail.tile([P, DV], F32, tag="ctx", bufs=1)
                for kc in range(KC):
                    nc.tensor.matmul(psc[:TW, :], attnT[:, kc, cs],
                                     v_sb[:, kc, :],
                                     start=(kc == 0), stop=(kc == KC - 1))
                ctx_sb = outp.tile([P, DV], F32, tag="ctx_sb")
                nc.vector.tensor_copy(ctx_sb[:TW, :], psc[:TW, :])
                nc.sync.dma_start(ctx_out[cs, :], ctx_sb[:TW, :])

            emitted_tail = 0
            row0 = 0
            for g, gr in enumerate(group_rows):
                rows = range(row0, row0 + gr)
                row0 += gr
                for hs in range(HC):
                    sums = sump.tile([P, G, K], F16, tag="sums")
                    for j, r in enumerate(rows):
                        nc.vector.tensor_scalar_add(
                            sums[:, j, :], kTp16[:, hs, :], qTp[:, hs, r:r + 1])
                    feat = featp.tile([P, G, K], F16, tag="feat")
                    nc.scalar.activation(feat[:, :gr, :], sums[:, :gr, :],
                                         mybir.ActivationFunctionType.Tanh)
                    for j, r in enumerate(rows):
                        for kc in range(KC):
                            nc.tensor.matmul(
                                score_ps[kc][:, hs, r:r + 1],
                                feat[:, j, kc * P:(kc + 1) * P],
                                wv16[:, hs:hs + 1],
                                start=True, stop=True)
                # emit finished tail quarters one group late so the DVE tail
                # work never stalls the next group's adds
                if (n_rows == QS and emitted_tail < 3
                        and row0 >= (emitted_tail + 1) * TW + G):
                    tail_part(emitted_tail)
                    emitted_tail += 1
            for t in range(emitted_tail, QS // TW):
                tail_part(t)


def build_nc(n_rows=QS):
    nc = bacc.Bacc("TRN2", target_bir_lowering=False, debug=False)
    ins = [
        nc.dram_tensor("query", [QS, D], F32, kind="ExternalInput").ap(),
        nc.dram_tensor("key", [K, D], F32, kind="ExternalInput").ap(),
        nc.dram_tensor("value", [K, DV], F32, kind="ExternalInput").ap(),
        nc.dram_tensor("Wq", [D, H], F32, kind="ExternalInput").ap(),
        nc.dram_tensor("bq", [H], F32, kind="ExternalInput").ap(),
        nc.dram_tensor("Wk", [D, H], F32, kind="ExternalInput").ap(),
        nc.dram_tensor("bk", [H], F32, kind="ExternalInput").ap(),
        nc.dram_tensor("wv", [H], F32, kind="ExternalInput").ap(),
    ]
    outs = [
        nc.dram_tensor("context", [QS, DV], F32, kind="ExternalOutput").ap(),
        nc.dram_tensor("attnT", [K, QS], F32, kind="ExternalOutput").ap(),
    ]
    with tile.TileContext(nc) as tc:
        _build_tile_kernel(tc, ins, outs, n_rows=n_rows)
    nc.compile()
    return nc


_NC_CACHE = None


def _get_nc():
    global _NC_CACHE
    if _NC_CACHE is None:
        _NC_CACHE = build_nc()
    return _NC_CACHE


def make_in_maps(query, key, value, Wq, bq, Wk, bk, wv):
    in_maps = []
    for c in range(N_CORES):
        b, half = c // 2, c % 2
        in_maps.append({
            "query": np.ascontiguousarray(query[b, half * QS:(half + 1) * QS, :]),
            "key": np.ascontiguousarray(key[b]),
            "value": np.ascontiguousarray(value[b]),
            "Wq": np.ascontiguousarray(Wq),
            "bq": np.ascontiguousarray(bq),
            "Wk": np.ascontiguousarray(Wk),
            "bk": np.ascontiguousarray(bk),
            "wv": np.ascontiguousarray(wv),
        })
    return in_maps


def gather_results(results):
    context = np.empty((B, Q, DV), np.float32)
    attn = np.empty((B, Q, K), np.float32)
    for c, r in enumerate(results):
        b, half = c // 2, c % 2
        context[b, half * QS:(half + 1) * QS, :] = r["context"]
        attn[b, half * QS:(half + 1) * QS, :] = np.ascontiguousarray(r["attnT"].T)
    return context, attn


def kernel(query, key, value, Wq, bq, Wk, bk, wv, bv, **run_kwargs):
    nc = _get_nc()
    in_maps = make_in_maps(
        np.asarray(query, np.float32), np.asarray(key, np.float32),
        np.asarray(value, np.float32), np.asarray(Wq, np.float32),
        np.asarray(bq, np.float32), np.asarray(Wk, np.float32),
        np.asarray(bk, np.float32), np.asarray(wv, np.float32))
    res = run_bass_kernel_spmd(nc, in_maps, core_ids=list(range(N_CORES)),
                               **run_kwargs)
    out = gather_results(res.results)
    if run_kwargs:
        return out, res
    return out


# revision 37
# speedup vs baseline: 1.0186x; 1.0186x over previous
"""Additive attention (Bahdanau) on 8 TRN2 NeuronCores.

Full-problem shapes: query [4,512,512], key/value [4,512,512],
Wq/Wk [512,256], bq/bk [256], wv [256], bv [].

  q = query @ Wq + bq                       # [B,Q,H]
  k = key @ Wk + bk                         # [B,K,H]
  score[b,q,k] = wv . tanh(q[b,q]+k[b,k])   # (+bv, dropped: softmax-invariant)
  attn = softmax(score, axis=-1)
  context = attn @ value

Sharding: data-parallel over (batch, query-half): core c handles batch c//2,
query rows (c%2)*256:(c%2+1)*256. Each core sees its full key/value batch, so
softmax is core-local; gather is pure numpy concatenation.

Per-core kernel layout: h (hidden) on partitions. For each query row r, the
scalar engine computes feat = tanh(kTp + qTp[:, r]) as one [128h, 512k]
activation per h-chunk (per-partition bias = q values), the tensor engine then
contracts with wv (feat stationary, wv the 1-column moving operand) writing a
scoreT[k-partition, r] column into PSUM. Softmax runs on the transposed scores
without any max-subtraction (|score| <= sum|wv| ~ 13, safe in fp32): exp on
the scalar engine, key-sum via ones-vector matmul over partitions, reciprocal
broadcast across partitions by a stride-0 DMA. The normalized attnT is
directly the lhsT of the context matmul. attnT is un-transposed on the host.
"""

import numpy as np

import concourse.bass as bass
import concourse.tile as tile
from concourse import bacc, mybir
from concourse.bass_utils import run_bass_kernel_spmd
from concourse.masks import make_identity

F32 = mybir.dt.float32
F16 = mybir.dt.float16

P = 128          # partitions
D = 512          # DQ = DK (projection input dim)
H = 256          # hidden dim; HC = H // P h-chunks
K = 512          # keys per batch; KC = K // P key chunks
QS = 256         # query rows per core
DV = 512         # value dim
HC, KC, DC, QT = H // P, K // P, D // P, QS // P

N_CORES = 8
B, Q = 4, 512


def _build_tile_kernel(tc, ins, outs, n_rows=QS):
    nc = tc.nc
    query, key, value, Wq, bq, Wk, bk, wv = ins
    ctx_out, attnT_out = outs

    raw_pool_cm = tc.tile_pool(name="raw", bufs=1)
    with tc.tile_pool(name="const", bufs=1) as const, \
         tc.tile_pool(name="proj", bufs=1) as proj, \
         tc.tile_pool(name="feat", bufs=2) as featp, \
         tc.tile_pool(name="tailp", bufs=1) as tailp, \
         tc.tile_pool(name="outp", bufs=2) as outp:

        raw = raw_pool_cm.__enter__()
        # ---- input DMAs, critical-path first: key, Wk, query, Wq -------
        k_raw = raw.tile([P, KC, D], F32)
        key_r = key.rearrange("(t p) d -> p t d", p=P)
        for t in range(KC):
            nc.sync.dma_start(k_raw[:, t, :], key_r[:, t, :])
        wk_sb = raw.tile([P, DC, H], F32)
        nc.sync.dma_start(wk_sb[:], Wk.rearrange("(c p) h -> p c h", p=P))
        q_raw = raw.tile([P, QT, D], F32)
        query_r = query.rearrange("(t p) d -> p t d", p=P)
        for t in range(QT):
            nc.sync.dma_start(q_raw[:, t, :], query_r[:, t, :])
        wq_sb = raw.tile([P, DC, H], F32)
        nc.sync.dma_start(wq_sb[:], Wq.rearrange("(c p) h -> p c h", p=P))
        # small/late tensors ride the gpsimd DMA queue, off the critical path
        bq_sb = const.tile([P, HC], F32)
        nc.gpsimd.dma_start(bq_sb[:], bq.rearrange("(o p) -> p o", p=P))
        bk_sb = const.tile([P, HC], F32)
        nc.gpsimd.dma_start(bk_sb[:], bk.rearrange("(o p) -> p o", p=P))
        wv32 = const.tile([P, HC], F32)
        nc.gpsimd.dma_start(wv32[:], wv.rearrange("(o p) -> p o", p=P))
        v_sb = const.tile([P, KC, DV], F32)   # only needed in the tail
        val_r = value.rearrange("(c p) v -> p c v", p=P)
        with tc.tile_wait_until(0.055):  # keep value traffic out of startup
            for t in range(KC):
                nc.gpsimd.dma_start(v_sb[:, t, :], val_r[:, t, :])

        ident = const.tile([P, P], F16)
        make_identity(nc, ident[:])
        # fp16 copies for transposes/projections on the (idle) scalar engine
        k16_raw = const.tile([P, KC, D], F16)
        for t in range(KC):
            nc.scalar.copy(k16_raw[:, t, :], k_raw[:, t, :])
        wk16 = const.tile([P, DC, H], F16)
        nc.scalar.copy(wk16[:], wk_sb[:])
        q16_raw = const.tile([P, QT, D], F16)
        for t in range(QT):
            nc.scalar.copy(q16_raw[:, t, :], q_raw[:, t, :])
        wq16 = const.tile([P, DC, H], F16)
        nc.scalar.copy(wq16[:], wq_sb[:])
        # off the critical path: wv cast + tail constants on gpsimd
        wv16 = const.tile([P, HC], F16)
        nc.gpsimd.tensor_copy(wv16[:], wv32[:])
        ones_sb = const.tile([P, 1], F32)     # k-sum matmul lhsT
        nc.gpsimd.memset(ones_sb[:], 1.0)
        ones_row = const.tile([1, P], F32)    # partition-broadcast via PE
        nc.gpsimd.memset(ones_row[:], 1.0)

        # ---- transpose query/key so d sits on partitions (fp16) --------
        qT = proj.tile([P, DC, QS], F16)      # [d_inner, d_chunk, q]
        kT = proj.tile([P, DC, K], F16)
        qTp = proj.tile([P, HC, QS], F32)     # fp32: feeds tensor_scalar adds
        kTp16 = proj.tile([P, HC, K], F16)
        with tc.tile_pool(name="ps_mm", bufs=2, space="PSUM") as ps_mm:
            for t in range(KC):
                for c in range(DC):
                    pst = ps_mm.tile([P, P], F16, tag="tp")
                    nc.tensor.transpose(pst[:], k16_raw[:, t, c * P:(c + 1) * P],
                                        ident[:])
                    nc.vector.tensor_copy(kT[:, c, t * P:(t + 1) * P], pst[:])
            for t in range(QT):
                for c in range(DC):
                    pst = ps_mm.tile([P, P], F16, tag="tp")
                    nc.tensor.transpose(pst[:], q16_raw[:, t, c * P:(c + 1) * P],
                                        ident[:])
                    nc.vector.tensor_copy(qT[:, c, t * P:(t + 1) * P], pst[:])

            # ---- projections, already transposed: [h, q] and [h, k] ----
            for hs in range(HC):
                psk = ps_mm.tile([P, K], F32, tag="psk", bufs=1)
                for c in range(DC):
                    nc.tensor.matmul(psk[:], wk16[:, c, hs * P:(hs + 1) * P],
                                     kT[:, c, :], start=(c == 0), stop=(c == DC - 1))
                nc.vector.tensor_scalar_add(kTp16[:, hs, :], psk[:],
                                            bk_sb[:, hs:hs + 1])
                psq = ps_mm.tile([P, QS], F32, tag="psq", bufs=1)
                for c in range(DC):
                    nc.tensor.matmul(psq[:], wq16[:, c, hs * P:(hs + 1) * P],
                                     qT[:, c, :], start=(c == 0), stop=(c == DC - 1))
                nc.vector.tensor_scalar_add(qTp[:, hs, :], psq[:],
                                            bq_sb[:, hs:hs + 1])

        raw_pool_cm.__exit__(None, None, None)

        # ---- main loop: per group, DVE adds -> one big tanh ->
        #      per-row wv matvecs into transposed-score PSUM columns.
        #      Small edge groups cut first-act latency and the last
        #      matvec burst before the tail. ----------------------------
        G = 16
        if n_rows == QS:
            group_rows = [4, 4, 8] + [16] * 14 + [8, 4, 4]
        else:
            group_rows = [min(G, n_rows - s0) for s0 in range(0, n_rows, G)]
        with tc.tile_pool(name="ps_score", bufs=1, space="PSUM") as ps_score, \
             tc.tile_pool(name="ps_tail", bufs=1, space="PSUM") as ps_tail, \
             tc.tile_pool(name="sump", bufs=2) as sump:
            score_ps = [ps_score.tile([P, HC, QS], F32, name=f"score_{kc}")
                        for kc in range(KC)]
            ssum = tailp.tile([P, KC, QS], F32)
            expT = tailp.tile([P, KC, QS], F32)
            sums_ps = ps_tail.tile([P, QS], F32, tag="sums")
            attnT = tailp.tile([P, KC, QS], F32)
            recipB = tailp.tile([P, QS], F32)

            TW = 64  # tail chunk width (query columns)

            def tail_part(t):
                """softmax + context for query columns [t*TW, (t+1)*TW)."""
                cs = slice(t * TW, (t + 1) * TW)
                for kc in range(KC):
                    nc.vector.tensor_reduce(
                        ssum[:, kc, cs],
                        score_ps[kc][:, :, cs].rearrange("p h c -> p c h"),
                        axis=mybir.AxisListType.X, op=mybir.AluOpType.add)
                for kc in range(KC):
                    nc.scalar.activation(expT[:, kc, cs], ssum[:, kc, cs],
                                         mybir.ActivationFunctionType.Exp)
                for kc in range(KC):
                    nc.tensor.matmul(sums_ps[0:1, cs], ones_sb[:],
                                     expT[:, kc, cs],
                                     start=(kc == 0), stop=(kc == KC - 1))
                sums_sb = tailp.tile([1, TW], F32, tag="sums_sb", bufs=2,
                                     name="sums_sb")
                nc.vector.tensor_copy(sums_sb[:], sums_ps[0:1, cs])
                # broadcast row across partitions via rank-1 PE outer product
                bc_ps = ps_tail.tile([P, TW], F32, tag="bc", bufs=1)
                nc.tensor.matmul(bc_ps[:], ones_row[:], sums_sb[:],
                                 start=True, stop=True)
                nc.vector.reciprocal(recipB[:, cs], bc_ps[:])
                nc.gpsimd.tensor_tensor(
                    attnT[:, :, cs], expT[:, :, cs],
                    recipB[:, None, cs].to_broadcast((P, KC, TW)),
                    mybir.AluOpType.mult)
                nc.sync.dma_start(
                    attnT_out.rearrange("(c p) q -> p c q", p=P)[:, :, cs],
                    attnT[:, :, cs])
                psc = ps_tail.tile([P, DV], F32, tag="ctx", bufs=1)
                for kc in range(KC):
                    nc.tensor.matmul(psc[:TW, :], attnT[:, kc, cs],
                                     v_sb[:, kc, :],
                                     start=(kc == 0), stop=(kc == KC - 1))
                ctx_sb = outp.tile([P, DV], F32, tag="ctx_sb")
                nc.vector.tensor_copy(ctx_sb[:TW, :], psc[:TW, :])
                nc.sync.dma_start(ctx_out[cs, :], ctx_sb[:TW, :])

            emitted_tail = 0
            row0 = 0
            for g, gr in enumerate(group_rows):
                rows = range(row0, row0 + gr)
                row0 += gr
                for hs in range(HC):
                    sums = sump.tile([P, G, K], F16, tag="sums")
                    for j, r in enumerate(rows):
                        nc.vector.tensor_scalar_add(
                            sums[:, j, :], kTp16[:, hs, :], qTp[:, hs, r:r + 1])
                    feat = featp.tile([P, G, K], F16, tag="feat")
                    nc.scalar.activation(feat[:, :gr, :], sums[:, :gr, :],
                                         mybir.ActivationFunctionType.Tanh)
                    for j, r in enumerate(rows):
                        for kc in range(KC):
                            nc.tensor.matmul(
                                score_ps[kc][:, hs, r:r + 1],
                                feat[:, j, kc * P:(kc + 1) * P],
                                wv16[:, hs:hs + 1],
                                start=True, stop=True)
                # emit finished tail quarters one group late so the DVE tail
                # work never stalls the next group's adds
                if (n_rows == QS and emitted_tail < 3
                        and row0 >= (emitted_tail + 1) * TW + G):
                    tail_part(emitted_tail)
                    emitted_tail += 1
            for t in range(emitted_tail, QS // TW):
                tail_part(t)


def build_nc(n_rows=QS):
    nc = bacc.Bacc("TRN2", target_bir_lowering=False, debug=False)
    ins = [
        nc.dram_tensor("query", [QS, D], F32, kind="ExternalInput").ap(),
        nc.dram_tensor("key", [K, D], F32, kind="ExternalInput").ap(),
        nc.dram_tensor("value", [K, DV], F32, kind="ExternalInput").ap(),
        nc.dram_tensor("Wq", [D, H], F32, kind="ExternalInput").ap(),
        nc.dram_tensor("bq", [H], F32, kind="ExternalInput").ap(),
        nc.dram_tensor("Wk", [D, H], F32, kind="ExternalInput").ap(),
        nc.dram_tensor("bk", [H], F32, kind="ExternalInput").ap(),
        nc.dram_tensor("wv", [H], F32, kind="ExternalInput").ap(),
    ]
    outs = [
        nc.dram_tensor("context", [QS, DV], F32, kind="ExternalOutput").ap(),
        nc.dram_tensor("attnT", [K, QS], F32, kind="ExternalOutput").ap(),
    ]
    with tile.TileContext(nc) as tc:
        _build_tile_kernel(tc, ins, outs, n_rows=n_rows)
    nc.compile()
    return nc


_NC_CACHE = None


def _get_nc():
    global _NC_CACHE
    if _NC_CACHE is None:
        _NC_CACHE = build_nc()
    return _NC_CACHE


def make_in_maps(query, key, value, Wq, bq, Wk, bk, wv):
    in_maps = []
    for c in range(N_CORES):
        b, half = c // 2, c % 2
        in_maps.append({
            "query": np.ascontiguousarray(query[b, half * QS:(half + 1) * QS, :]),
            "key": np.ascontiguousarray(key[b]),
            "value": np.ascontiguousarray(value[b]),
            "Wq": np.ascontiguousarray(Wq),
            "bq": np.ascontiguousarray(bq),
            "Wk": np.ascontiguousarray(Wk),
            "bk": np.ascontiguousarray(bk),
            "wv": np.ascontiguousarray(wv),
        })
    return in_maps


def gather_results(results):
    context = np.empty((B, Q, DV), np.float32)
    attn = np.empty((B, Q, K), np.float32)
    for c, r in enumerate(results):
        b, half = c // 2, c % 2
        context[b, half * QS:(half + 1) * QS, :] = r["context"]
        attn[b, half * QS:(half + 1) * QS, :] = np.ascontiguousarray(r["attnT"].T)
    return context, attn


def kernel(query, key, value, Wq, bq, Wk, bk, wv, bv, **run_kwargs):
    nc = _get_nc()
    in_maps = make_in_maps(
        np.asarray(query, np.float32), np.asarray(key, np.float32),
        np.asarray(value, np.float32), np.asarray(Wq, np.float32),
        np.asarray(bq, np.float32), np.asarray(Wk, np.float32),
        np.asarray(bk, np.float32), np.asarray(wv, np.float32))
    res = run_bass_kernel_spmd(nc, in_maps, core_ids=list(range(N_CORES)),
                               **run_kwargs)
    out = gather_results(res.results)
    if run_kwargs:
        return out, res
    return out


# revision 38
# speedup vs baseline: 1.0414x; 1.0223x over previous
"""Additive attention (Bahdanau) on 8 TRN2 NeuronCores.

Full-problem shapes: query [4,512,512], key/value [4,512,512],
Wq/Wk [512,256], bq/bk [256], wv [256], bv [].

  q = query @ Wq + bq                       # [B,Q,H]
  k = key @ Wk + bk                         # [B,K,H]
  score[b,q,k] = wv . tanh(q[b,q]+k[b,k])   # (+bv, dropped: softmax-invariant)
  attn = softmax(score, axis=-1)
  context = attn @ value

Sharding: data-parallel over (batch, query-half): core c handles batch c//2,
query rows (c%2)*256:(c%2+1)*256. Each core sees its full key/value batch, so
softmax is core-local; gather is pure numpy concatenation.

Per-core kernel layout: h (hidden) on partitions. For each query row r, the
scalar engine computes feat = tanh(kTp + qTp[:, r]) as one [128h, 512k]
activation per h-chunk (per-partition bias = q values), the tensor engine then
contracts with wv (feat stationary, wv the 1-column moving operand) writing a
scoreT[k-partition, r] column into PSUM. Softmax runs on the transposed scores
without any max-subtraction (|score| <= sum|wv| ~ 13, safe in fp32): exp on
the scalar engine, key-sum via ones-vector matmul over partitions, reciprocal
broadcast across partitions by a stride-0 DMA. The normalized attnT is
directly the lhsT of the context matmul. attnT is un-transposed on the host.
"""

import numpy as np

import concourse.bass as bass
import concourse.tile as tile
from concourse import bacc, mybir
from concourse.bass_utils import run_bass_kernel_spmd
from concourse.masks import make_identity

F32 = mybir.dt.float32
F16 = mybir.dt.float16

P = 128          # partitions
D = 512          # DQ = DK (projection input dim)
H = 256          # hidden dim; HC = H // P h-chunks
K = 512          # keys per batch; KC = K // P key chunks
QS = 256         # query rows per core
DV = 512         # value dim
HC, KC, DC, QT = H // P, K // P, D // P, QS // P

N_CORES = 8
B, Q = 4, 512


def _build_tile_kernel(tc, ins, outs, n_rows=QS):
    nc = tc.nc
    query, key, value, Wq, bq, Wk, bk, wv = ins
    ctx_out, attnT_out = outs

    raw_pool_cm = tc.tile_pool(name="raw", bufs=1)
    with tc.tile_pool(name="const", bufs=1) as const, \
         tc.tile_pool(name="proj", bufs=1) as proj, \
         tc.tile_pool(name="feat", bufs=2) as featp, \
         tc.tile_pool(name="tailp", bufs=1) as tailp, \
         tc.tile_pool(name="outp", bufs=2) as outp:

        raw = raw_pool_cm.__enter__()
        # ---- input DMAs, critical-path first: key, Wk, query, Wq -------
        k_raw = raw.tile([P, KC, D], F32)
        key_r = key.rearrange("(t p) d -> p t d", p=P)
        for t in range(KC):
            nc.sync.dma_start(k_raw[:, t, :], key_r[:, t, :])
        wk_sb = raw.tile([P, DC, H], F32)
        nc.sync.dma_start(wk_sb[:], Wk.rearrange("(c p) h -> p c h", p=P))
        q_raw = raw.tile([P, QT, D], F32)
        query_r = query.rearrange("(t p) d -> p t d", p=P)
        for t in range(QT):
            nc.sync.dma_start(q_raw[:, t, :], query_r[:, t, :])
        wq_sb = raw.tile([P, DC, H], F32)
        nc.sync.dma_start(wq_sb[:], Wq.rearrange("(c p) h -> p c h", p=P))
        # small/late tensors ride the gpsimd DMA queue, off the critical path
        bq_sb = const.tile([P, HC], F32)
        nc.gpsimd.dma_start(bq_sb[:], bq.rearrange("(o p) -> p o", p=P))
        bk_sb = const.tile([P, HC], F32)
        nc.gpsimd.dma_start(bk_sb[:], bk.rearrange("(o p) -> p o", p=P))
        wv32 = const.tile([P, HC], F32)
        nc.gpsimd.dma_start(wv32[:], wv.rearrange("(o p) -> p o", p=P))
        v_sb = const.tile([P, KC, DV], F32)   # only needed in the tail
        val_r = value.rearrange("(c p) v -> p c v", p=P)
        with tc.tile_wait_until(0.055):  # keep value traffic out of startup
            for t in range(KC):
                nc.gpsimd.dma_start(v_sb[:, t, :], val_r[:, t, :])

        ident = const.tile([P, P], F16)
        make_identity(nc, ident[:])
        # fp16 copies for transposes/projections on the (idle) scalar engine
        k16_raw = const.tile([P, KC, D], F16)
        for t in range(KC):
            nc.scalar.copy(k16_raw[:, t, :], k_raw[:, t, :])
        wk16 = const.tile([P, DC, H], F16)
        nc.scalar.copy(wk16[:], wk_sb[:])
        q16_raw = const.tile([P, QT, D], F16)
        for t in range(QT):
            nc.scalar.copy(q16_raw[:, t, :], q_raw[:, t, :])
        wq16 = const.tile([P, DC, H], F16)
        nc.scalar.copy(wq16[:], wq_sb[:])
        # off the critical path: wv cast + tail constants on gpsimd
        wv16 = const.tile([P, HC], F16)
        nc.gpsimd.tensor_copy(wv16[:], wv32[:])
        ones_sb = const.tile([P, 1], F32)     # k-sum matmul lhsT
        nc.gpsimd.memset(ones_sb[:], 1.0)
        ones_row = const.tile([1, P], F32)    # partition-broadcast via PE
        nc.gpsimd.memset(ones_row[:], 1.0)
        v16 = const.tile([P, KC, DV], F16)    # chunk casts emitted mid-loop

        # ---- transpose query/key so d sits on partitions (fp16) --------
        qT = proj.tile([P, DC, QS], F16)      # [d_inner, d_chunk, q]
        kT = proj.tile([P, DC, K], F16)
        qTp = proj.tile([P, HC, QS], F32)     # fp32: feeds tensor_scalar adds
        kTp16 = proj.tile([P, HC, K], F16)
        with tc.tile_pool(name="ps_mm", bufs=2, space="PSUM") as ps_mm:
            for t in range(KC):
                for c in range(DC):
                    pst = ps_mm.tile([P, P], F16, tag="tp")
                    nc.tensor.transpose(pst[:], k16_raw[:, t, c * P:(c + 1) * P],
                                        ident[:])
                    nc.vector.tensor_copy(kT[:, c, t * P:(t + 1) * P], pst[:])
            for t in range(QT):
                for c in range(DC):
                    pst = ps_mm.tile([P, P], F16, tag="tp")
                    nc.tensor.transpose(pst[:], q16_raw[:, t, c * P:(c + 1) * P],
                                        ident[:])
                    nc.vector.tensor_copy(qT[:, c, t * P:(t + 1) * P], pst[:])

            # ---- projections, already transposed: [h, q] and [h, k] ----
            for hs in range(HC):
                psk = ps_mm.tile([P, K], F32, tag="psk", bufs=1)
                for c in range(DC):
                    nc.tensor.matmul(psk[:], wk16[:, c, hs * P:(hs + 1) * P],
                                     kT[:, c, :], start=(c == 0), stop=(c == DC - 1))
                nc.vector.tensor_scalar_add(kTp16[:, hs, :], psk[:],
                                            bk_sb[:, hs:hs + 1])
                psq = ps_mm.tile([P, QS], F32, tag="psq", bufs=1)
                for c in range(DC):
                    nc.tensor.matmul(psq[:], wq16[:, c, hs * P:(hs + 1) * P],
                                     qT[:, c, :], start=(c == 0), stop=(c == DC - 1))
                nc.vector.tensor_scalar_add(qTp[:, hs, :], psq[:],
                                            bq_sb[:, hs:hs + 1])

        raw_pool_cm.__exit__(None, None, None)

        # ---- main loop: per group, DVE adds -> one big tanh ->
        #      per-row wv matvecs into transposed-score PSUM columns.
        #      Small edge groups cut first-act latency and the last
        #      matvec burst before the tail. ----------------------------
        G = 16
        if n_rows == QS:
            group_rows = [4, 4, 8] + [16] * 14 + [8, 4, 4]
        else:
            group_rows = [min(G, n_rows - s0) for s0 in range(0, n_rows, G)]
        with tc.tile_pool(name="ps_score", bufs=1, space="PSUM") as ps_score, \
             tc.tile_pool(name="ps_tail", bufs=1, space="PSUM") as ps_tail, \
             tc.tile_pool(name="sump", bufs=2) as sump:
            score_ps = [ps_score.tile([P, HC, QS], F32, name=f"score_{kc}")
                        for kc in range(KC)]
            ssum = tailp.tile([P, KC, QS], F32)
            expT = tailp.tile([P, KC, QS], F32)
            sums_ps = ps_tail.tile([P, QS], F32, tag="sums")
            attnT = tailp.tile([P, KC, QS], F32)
            recipB = tailp.tile([P, QS], F32)

            TW = 64  # tail chunk width (query columns)

            def tail_part(t):
                """softmax + context for query columns [t*TW, (t+1)*TW)."""
                cs = slice(t * TW, (t + 1) * TW)
                for kc in range(KC):
                    nc.vector.tensor_reduce(
                        ssum[:, kc, cs],
                        score_ps[kc][:, :, cs].rearrange("p h c -> p c h"),
                        axis=mybir.AxisListType.X, op=mybir.AluOpType.add)
                for kc in range(KC):
                    nc.scalar.activation(expT[:, kc, cs], ssum[:, kc, cs],
                                         mybir.ActivationFunctionType.Exp)
                for kc in range(KC):
                    nc.tensor.matmul(sums_ps[0:1, cs], ones_sb[:],
                                     expT[:, kc, cs],
                                     start=(kc == 0), stop=(kc == KC - 1))
                sums_sb = tailp.tile([1, TW], F32, tag="sums_sb", bufs=2,
                                     name="sums_sb")
                nc.vector.tensor_copy(sums_sb[:], sums_ps[0:1, cs])
                # broadcast row across partitions via rank-1 PE outer product
                bc_ps = ps_tail.tile([P, TW], F32, tag="bc", bufs=1)
                nc.tensor.matmul(bc_ps[:], ones_row[:], sums_sb[:],
                                 start=True, stop=True)
                nc.vector.reciprocal(recipB[:, cs], bc_ps[:])
                nc.gpsimd.tensor_tensor(
                    attnT[:, :, cs], expT[:, :, cs],
                    recipB[:, None, cs].to_broadcast((P, KC, TW)),
                    mybir.AluOpType.mult)
                attnT16 = tailp.tile([P, KC, TW], F16, tag="attnT16", bufs=2,
                                     name="attnT16")
                nc.gpsimd.tensor_tensor(
                    attnT16[:], expT[:, :, cs],
                    recipB[:, None, cs].to_broadcast((P, KC, TW)),
                    mybir.AluOpType.mult)
                nc.sync.dma_start(
                    attnT_out.rearrange("(c p) q -> p c q", p=P)[:, :, cs],
                    attnT[:, :, cs])
                psc = ps_tail.tile([P, DV], F32, tag="ctx", bufs=1)
                for kc in range(KC):
                    nc.tensor.matmul(psc[:TW, :], attnT16[:, kc, :],
                                     v16[:, kc, :],
                                     start=(kc == 0), stop=(kc == KC - 1))
                ctx_sb = outp.tile([P, DV], F32, tag="ctx_sb")
                nc.vector.tensor_copy(ctx_sb[:TW, :], psc[:TW, :])
                nc.sync.dma_start(ctx_out[cs, :], ctx_sb[:TW, :])

            emitted_tail = 0
            row0 = 0
            for g, gr in enumerate(group_rows):
                rows = range(row0, row0 + gr)
                row0 += gr
                for hs in range(HC):
                    sums = sump.tile([P, G, K], F16, tag="sums")
                    for j, r in enumerate(rows):
                        nc.vector.tensor_scalar_add(
                            sums[:, j, :], kTp16[:, hs, :], qTp[:, hs, r:r + 1])
                    feat = featp.tile([P, G, K], F16, tag="feat")
                    nc.scalar.activation(feat[:, :gr, :], sums[:, :gr, :],
                                         mybir.ActivationFunctionType.Tanh)
                    for j, r in enumerate(rows):
                        for kc in range(KC):
                            nc.tensor.matmul(
                                score_ps[kc][:, hs, r:r + 1],
                                feat[:, j, kc * P:(kc + 1) * P],
                                wv16[:, hs:hs + 1],
                                start=True, stop=True)
                if n_rows == QS and 3 <= g <= 6:
                    nc.vector.tensor_copy(v16[:, g - 3, :], v_sb[:, g - 3, :])
                # emit finished tail quarters one group late so the DVE tail
                # work never stalls the next group's adds
                if (n_rows == QS and emitted_tail < 3
                        and row0 >= (emitted_tail + 1) * TW + G):
                    tail_part(emitted_tail)
                    emitted_tail += 1
            for t in range(emitted_tail, QS // TW):
                tail_part(t)


def build_nc(n_rows=QS):
    nc = bacc.Bacc("TRN2", target_bir_lowering=False, debug=False)
    ins = [
        nc.dram_tensor("query", [QS, D], F32, kind="ExternalInput").ap(),
        nc.dram_tensor("key", [K, D], F32, kind="ExternalInput").ap(),
        nc.dram_tensor("value", [K, DV], F32, kind="ExternalInput").ap(),
        nc.dram_tensor("Wq", [D, H], F32, kind="ExternalInput").ap(),
        nc.dram_tensor("bq", [H], F32, kind="ExternalInput").ap(),
        nc.dram_tensor("Wk", [D, H], F32, kind="ExternalInput").ap(),
        nc.dram_tensor("bk", [H], F32, kind="ExternalInput").ap(),
        nc.dram_tensor("wv", [H], F32, kind="ExternalInput").ap(),
    ]
    outs = [
        nc.dram_tensor("context", [QS, DV], F32, kind="ExternalOutput").ap(),
        nc.dram_tensor("attnT", [K, QS], F32, kind="ExternalOutput").ap(),
    ]
    with tile.TileContext(nc) as tc:
        _build_tile_kernel(tc, ins, outs, n_rows=n_rows)
    nc.compile()
    return nc


_NC_CACHE = None


def _get_nc():
    global _NC_CACHE
    if _NC_CACHE is None:
        _NC_CACHE = build_nc()
    return _NC_CACHE


def make_in_maps(query, key, value, Wq, bq, Wk, bk, wv):
    in_maps = []
    for c in range(N_CORES):
        b, half = c // 2, c % 2
        in_maps.append({
            "query": np.ascontiguousarray(query[b, half * QS:(half + 1) * QS, :]),
            "key": np.ascontiguousarray(key[b]),
            "value": np.ascontiguousarray(value[b]),
            "Wq": np.ascontiguousarray(Wq),
            "bq": np.ascontiguousarray(bq),
            "Wk": np.ascontiguousarray(Wk),
            "bk": np.ascontiguousarray(bk),
            "wv": np.ascontiguousarray(wv),
        })
    return in_maps


def gather_results(results):
    context = np.empty((B, Q, DV), np.float32)
    attn = np.empty((B, Q, K), np.float32)
    for c, r in enumerate(results):
        b, half = c // 2, c % 2
        context[b, half * QS:(half + 1) * QS, :] = r["context"]
        attn[b, half * QS:(half + 1) * QS, :] = np.ascontiguousarray(r["attnT"].T)
    return context, attn


def kernel(query, key, value, Wq, bq, Wk, bk, wv, bv, **run_kwargs):
    nc = _get_nc()
    in_maps = make_in_maps(
        np.asarray(query, np.float32), np.asarray(key, np.float32),
        np.asarray(value, np.float32), np.asarray(Wq, np.float32),
        np.asarray(bq, np.float32), np.asarray(Wk, np.float32),
        np.asarray(bk, np.float32), np.asarray(wv, np.float32))
    res = run_bass_kernel_spmd(nc, in_maps, core_ids=list(range(N_CORES)),
                               **run_kwargs)
    out = gather_results(res.results)
    if run_kwargs:
        return out, res
    return out
